# revision 7
# baseline (speedup 1.0000x reference)
"""Trainium2 Bass kernel for nn_CrossAttentionDecoder (B=4, S=1024, D=1024, H=4096, 16 heads).

Sharding: 8 cores, zero collectives. Core c = 2*b + half handles batch element b
and query-row half `half` (512 rows). Each core computes self/cross K,V for the
full sequence of its batch element (redundant across the pair of cores sharing a
batch element) so no cross-core communication is needed.

The host rotates each core's x so its own query rows are always rows 0:512 —
attention here is permutation-invariant over key order, so one SPMD NEFF serves
all cores. LN gains (g1, g2) are folded into the following weight matrices on
the host (exact); biases are applied on-chip (per-partition ACT biases in
transposed layouts, ones-row rank-1 matmuls in row-major layouts).

Layout strategy: activations move through the matmul pipeline feature-on-
partitions ("transposed") so softmax reductions are free-axis; attention
probability matrices are never transposed. Softmax denominators come from an
appended ones-column in V (self-attn) / ones-column rhs matmuls (cross-attn),
and normalization is applied where the per-row scale is a per-partition scalar.
"""

import os
import sys
from contextlib import ExitStack

import numpy as np

for _p in ("/opt/trn_rl_repo",):
    if _p not in sys.path and os.path.isdir(_p):
        sys.path.append(_p)

import ml_dtypes
import concourse.bass as bass
import concourse.mybir as mybir
import concourse.tile as tile
from concourse.bass_utils import run_bass_kernel_spmd
from concourse.masks import make_identity

BF16 = mybir.dt.bfloat16
FP32 = mybir.dt.float32
AF = mybir.ActivationFunctionType

S = 1024          # full sequence (keys)
R = 512           # query rows owned per core
D = 1024          # model dim
H = 4096          # mlp hidden
NH = 16           # self-attn heads
P = 128           # partitions
KC = D // P       # 8 contraction chunks over D
RC = R // P       # 4 row chunks
HC = H // P       # 32 chunks over H
EPS = 1e-5


# ---------------------------------------------------------------------------
# Workaround: this walrus build accepts only one sync-wait per CTRL
# instruction; TileContext's final drain can carry several.  Spread them
# across single-wait SP nops emitted just before the drain.
def _patched_drain_and_barrier(self, tick_clock, wait_clock):
    from concourse.vector_clock import ScopedClock

    nc = self.nc
    collector = nc.sync.nop(nofuse=True, hint="drain_wait_collector")
    wait_clock.add_sem_waits(
        collector.ins, ScopedClock({None: tick_clock.global_clock})
    )
    si = collector.ins.sync_info
    waits = list(si.on_wait) if si is not None else []
    if si is not None and len(waits) > 1:
        si.on_wait = waits[:1]
        for w in waits[1:]:
            extra = nc.sync.nop(nofuse=True, hint="drain_wait_extra")
            if extra.ins.sync_info is None:
                extra.ins.sync_info = mybir.SyncInfo(on_wait=[w], on_update=[])
            else:
                extra.ins.sync_info.on_wait = [w]

    nc.sync.drain()
    nc.all_engine_barrier()
    assert self.sems is not None
    popped = nc._tile_sem_poison_stack.pop()
    assert popped is self._sem_poison
    nc.clear_and_free_semaphores(list(self.sems.allocated().values()))
    nc.all_engine_barrier()


tile.TileContext._drain_and_barrier = _patched_drain_and_barrier


def _split_multi_waits(nc):
    """Same walrus limitation, applied module-wide: any instruction carrying
    more than one sync wait gets its extra waits moved onto single-wait nops
    emitted just before it on the same engine."""
    n = 0
    for func in nc.m.functions:
        for bb in func.blocks:
            new_insts = []
            for inst in bb.instructions:
                si = inst.sync_info
                if si is not None and len(si.on_wait) > 1:
                    waits = list(si.on_wait)
                    for w in waits[:-1]:
                        n += 1
                        new_insts.append(mybir.InstNoOp(
                            name=f"wsplit-{n}",
                            engine=inst.engine,
                            bass_nofuse=True,
                            sync_info=mybir.SyncInfo(on_wait=[w], on_update=[]),
                        ))
                    si.on_wait = [waits[-1]]
                new_insts.append(inst)
            try:
                bb.instructions = new_insts
            except AttributeError:
                bb.instructions.clear()
                bb.instructions.extend(new_insts)
    return n
# ---------------------------------------------------------------------------


def build():
    nc = bass.Bass("TRN2")

    x_d = nc.dram_tensor("x", [S, D], FP32, kind="ExternalInput")
    z_d = nc.dram_tensor("z", [S, D], BF16, kind="ExternalInput")
    wqkv_d = nc.dram_tensor("wqkv", [D, 3 * D], BF16, kind="ExternalInput")
    wo_d = nc.dram_tensor("wo", [D, D], BF16, kind="ExternalInput")
    wkv_d = nc.dram_tensor("wkv", [D, 2 * D], BF16, kind="ExternalInput")
    wm1_d = nc.dram_tensor("wm1", [D, H], BF16, kind="ExternalInput")
    wm2_d = nc.dram_tensor("wm2", [H, D], BF16, kind="ExternalInput")
    bqkv_d = nc.dram_tensor("bqkv", [3 * D], FP32, kind="ExternalInput")
    bkv_d = nc.dram_tensor("bkv", [2 * D], FP32, kind="ExternalInput")
    bo_d = nc.dram_tensor("bo", [D], FP32, kind="ExternalInput")
    bm1_d = nc.dram_tensor("bm1", [H], FP32, kind="ExternalInput")
    bm2_d = nc.dram_tensor("bm2", [D], FP32, kind="ExternalInput")
    out_d = nc.dram_tensor("out", [R, D], FP32, kind="ExternalOutput")

    with tile.TileContext(nc) as tc, ExitStack() as top:
        # ----- persistent small constants ---------------------------------
        const = top.enter_context(tc.tile_pool(name="const", bufs=1))
        ident = const.tile([P, P], BF16, tag="ident")
        make_identity(nc, ident[:])
        eps_t = const.tile([P, 1], FP32, tag="eps")
        nc.vector.memset(eps_t[:], EPS)
        ones_f32_row = const.tile([1, P], FP32, tag="ones_f32")  # bias rank-1 lhsT
        nc.vector.memset(ones_f32_row[:], 1.0)
        ones_p64 = const.tile([65, 64], FP32, tag="ones_p64")    # partition-64 row
        nc.vector.memset(ones_p64[64:65, :], 1.0)
        ones_bf_col = const.tile([P, 1], BF16, tag="ones_bf_col")  # cross denom rhs
        nc.vector.memset(ones_bf_col[:], 1.0)

        # bias vectors
        bq_pm = const.tile([P, KC], FP32, tag="bq_pm")        # q bias, per-partition
        nc.sync.dma_start(bq_pm[:], bqkv_d[0:D].rearrange("(c p) -> p c", p=P))
        bk_pm = const.tile([P, KC], FP32, tag="bk_pm")        # k bias, per-partition
        nc.sync.dma_start(bk_pm[:], bqkv_d[D:2 * D].rearrange("(c p) -> p c", p=P))
        bv_row = const.tile([1, D], FP32, tag="bv_row")       # v bias, row layout
        nc.sync.dma_start(bv_row[:], bqkv_d[2 * D:3 * D][None, :])
        bkc_pm = const.tile([P, KC], FP32, tag="bkc_pm")      # cross k bias
        nc.sync.dma_start(bkc_pm[:], bkv_d[0:D].rearrange("(c p) -> p c", p=P))
        bvc_row = const.tile([1, D], FP32, tag="bvc_row")     # cross v bias
        nc.sync.dma_start(bvc_row[:], bkv_d[D:2 * D][None, :])
        bo_row = const.tile([1, D], FP32, tag="bo_row")
        nc.sync.dma_start(bo_row[:], bo_d[None, :])
        bm1_pm = const.tile([P, HC], FP32, tag="bm1_pm")
        nc.sync.dma_start(bm1_pm[:], bm1_d.rearrange("(c p) -> p c", p=P))
        bm2_row = const.tile([1, D], FP32, tag="bm2_row")
        nc.sync.dma_start(bm2_row[:], bm2_d[None, :])

        # ----- long-lived activation pools.  Pools reserve their footprint
        # for their whole open span and must close LIFO, so opens are ordered
        # by decreasing close time; big late-stage pools (h1T/ln2T) open late.
        outc_cm = tc.tile_pool(name="outc", bufs=1)            # S6..S9
        outc_pool = outc_cm.__enter__()
        qT_cm = tc.tile_pool(name="qT", bufs=1)                # S4..S6
        qT_pool = qT_cm.__enter__()
        x_cm = tc.tile_pool(name="x_own", bufs=1)              # S1..S4
        x_pool = x_cm.__enter__()
        att_cm = tc.tile_pool(name="attT", bufs=1)             # S3..S4
        att_pool = att_cm.__enter__()
        lnT_cm = tc.tile_pool(name="lnT", bufs=1)              # S1..S3
        lnT_pool = lnT_cm.__enter__()
        qkv_cm = tc.tile_pool(name="qkvT", bufs=1)             # S2..S3
        qkv_pool = qkv_cm.__enter__()

        x_own = x_pool.tile([P, RC, D], FP32, tag="x_own")     # 2 MiB (own rows)
        lnT = lnT_pool.tile([P, KC, S], BF16, tag="lnT")       # 2 MiB

        # ================= Stage 1: LN1 + transpose =======================
        with tc.tile_pool(name="s1", bufs=3) as s1, \
             tc.tile_pool(name="s1ps", bufs=4, space="PSUM") as s1ps:
            for i in range(KC):  # 8 row-chunks of x; rows 0:R are owned
                if i < RC:
                    xt = x_own[:, i, :]
                else:
                    xts = s1.tile([P, D], FP32, tag="xt")
                    xt = xts[:]
                nc.sync.dma_start(xt, x_d[i * P:(i + 1) * P, :])
                stats = s1.tile([P, 2, 6], FP32, tag="stats")
                nc.vector.bn_stats(stats[:, 0, :], xt[:, 0:512])
                nc.vector.bn_stats(stats[:, 1, :], xt[:, 512:1024])
                mv = s1.tile([P, 2], FP32, tag="mv")
                nc.vector.bn_aggr(mv[:], stats[:])
                rstd = s1.tile([P, 1], FP32, tag="rstd")
                nc.scalar.activation(rstd[:], mv[:, 1:2], AF.Sqrt, bias=eps_t[:])
                nc.vector.reciprocal(rstd[:], rstd[:])
                ln_row = s1.tile([P, D], BF16, tag="ln_row")
                nc.vector.tensor_scalar(
                    out=ln_row[:], in0=xt,
                    scalar1=mv[:, 0:1], scalar2=rstd[:],
                    op0=mybir.AluOpType.subtract, op1=mybir.AluOpType.mult,
                )
                for j in range(KC):
                    tp = s1ps.tile([P, P], BF16, tag="tp")
                    nc.tensor.transpose(tp[:], ln_row[:, j * P:(j + 1) * P], ident[:])
                    nc.vector.tensor_copy(lnT[:, j, i * P:(i + 1) * P], tp[:])

        # ================= Stage 2: QKV projections =======================
        QT = qkv_pool.tile([P, KC, R], BF16, tag="QT")         # 1 MiB
        KT = qkv_pool.tile([P, KC, S], BF16, tag="KT")         # 2 MiB
        V_aug = qkv_pool.tile([P, KC, NH * 65], BF16, tag="V_aug")
        for h in range(NH):  # ones column at h*65+64 -> free softmax denominator
            nc.vector.memset(V_aug[:, :, h * 65 + 64], 1.0)

        with tc.tile_pool(name="wqkv", bufs=1) as wq_pool, \
             tc.tile_pool(name="s2ps", bufs=4, space="PSUM") as s2ps:
            wq_sb = wq_pool.tile([P, KC, 3 * D], BF16, tag="wq_sb")  # 6 MiB
            for k in range(KC):
                nc.sync.dma_start(wq_sb[:, k, :], wqkv_d[k * P:(k + 1) * P, :])
            # --- Q^T (own rows only) ---
            for m in range(KC):
                ps = s2ps.tile([P, R], FP32, tag="mm")
                for k in range(KC):
                    nc.tensor.matmul(
                        ps[:], wq_sb[:, k, m * P:(m + 1) * P], lnT[:, k, 0:R],
                        start=(k == 0), stop=(k == KC - 1))
                nc.scalar.activation(QT[:, m, :], ps[:], AF.Identity,
                                     bias=bq_pm[:, m:m + 1])
            # --- K^T (full seq) ---
            for m in range(KC):
                for n in range(2):
                    ps = s2ps.tile([P, R], FP32, tag="mm")
                    for k in range(KC):
                        nc.tensor.matmul(
                            ps[:], wq_sb[:, k, D + m * P:D + (m + 1) * P],
                            lnT[:, k, n * 512:(n + 1) * 512],
                            start=(k == 0), stop=(k == KC - 1))
                    nc.scalar.activation(KT[:, m, n * 512:(n + 1) * 512], ps[:],
                                         AF.Identity, bias=bk_pm[:, m:m + 1])
            # --- V (row-major, strided into V_aug between the ones cols) ---
            for si in range(KC):
                for n in range(2):
                    ps = s2ps.tile([P, R], FP32, tag="mm")
                    for k in range(KC):
                        nc.tensor.matmul(
                            ps[:], lnT[:, k, si * P:(si + 1) * P],
                            wq_sb[:, k, 2 * D + n * 512:2 * D + (n + 1) * 512],
                            start=(k == 0), stop=False)
                    # bias: out += ones[s] (x) bv[n-slice]  (fp32 rank-1 matmul)
                    nc.tensor.matmul(
                        ps[:], ones_f32_row[:, 0:P],
                        bv_row[:, n * 512:(n + 1) * 512],
                        start=False, stop=True, skip_group_check=True)
                    va = V_aug[:, si, :].rearrange("p (h c) -> p h c", c=65)
                    nc.vector.tensor_copy(
                        va[:, n * 8:(n + 1) * 8, 0:64],
                        ps[:].rearrange("p (h c) -> p h c", c=64))

        # ================= Stage 3: self-attention ========================
        attT = att_pool.tile([P, KC, R], BF16, tag="attT")     # 1 MiB

        with tc.tile_pool(name="s3", bufs=4) as s3, \
             tc.tile_pool(name="s3r", bufs=4) as s3r, \
             tc.tile_pool(name="s3ps", bufs=3, space="PSUM") as s3ps, \
             tc.tile_pool(name="s3po", bufs=2, space="PSUM") as s3po, \
             tc.tile_pool(name="s3pb", bufs=2, space="PSUM") as s3pb:
            for h in range(NH):
                kc, off = h // 2, (h % 2) * 64
                po = s3po.tile([65, R], FP32, tag="po")
                for si in range(KC):
                    ps = s3ps.tile([P, R], FP32, tag="sc")
                    nc.tensor.matmul(
                        ps[:], KT[off:off + 64, kc, si * P:(si + 1) * P],
                        QT[off:off + 64, kc, :], start=True, stop=True)
                    e = s3.tile([P, R], BF16, tag="e")
                    nc.scalar.activation(e[:], ps[:], AF.Exp, scale=0.125)
                    nc.tensor.matmul(
                        po[:], V_aug[:, si, h * 65:h * 65 + 65], e[:],
                        start=(si == 0), stop=(si == KC - 1))
                # reciprocal of the denominator row (partition 64 throughout)
                recip = s3r.tile([65, R], FP32, tag="recip")
                nc.vector.reciprocal(recip[64:65, :], po[64:65, :])
                pb = s3pb.tile([64, R], FP32, tag="pb")
                nc.tensor.matmul(pb[:], ones_p64[64:65, :], recip[64:65, :],
                                 start=True, stop=True)
                pb_sb = s3r.tile([64, R], FP32, tag="pb_sb")
                nc.scalar.copy(pb_sb[:], pb[:])
                if off == 0:
                    nc.vector.tensor_mul(attT[0:64, kc, :], po[0:64, :], pb_sb[:])
                else:
                    stg = s3r.tile([64, R], BF16, tag="stg")
                    nc.vector.tensor_mul(stg[:], po[0:64, :], pb_sb[:])
                    nc.sync.dma_start(attT[64:128, kc, :], stg[:])

        qkv_cm.__exit__(None, None, None)
        lnT_cm.__exit__(None, None, None)

        # ================= Stage 4: out-proj + residual + qT ==============
        qT = qT_pool.tile([P, KC, R], BF16, tag="qT")          # 1 MiB

        with tc.tile_pool(name="wo", bufs=1) as wo_pool, \
             tc.tile_pool(name="s4", bufs=3) as s4, \
             tc.tile_pool(name="s4ps", bufs=3, space="PSUM") as s4ps, \
             tc.tile_pool(name="s4pt", bufs=3, space="PSUM") as s4pt:
            wo_sb = wo_pool.tile([P, KC, D], BF16, tag="wo_sb")  # 2 MiB
            for k in range(KC):
                nc.sync.dma_start(wo_sb[:, k, :], wo_d[k * P:(k + 1) * P, :])
            for rm in range(RC):
                qrow = s4.tile([P, D], BF16, tag="qrow")
                for n in range(2):
                    ps = s4ps.tile([P, 512], FP32, tag="mm")
                    for k in range(KC):
                        nc.tensor.matmul(
                            ps[:], attT[:, k, rm * P:(rm + 1) * P],
                            wo_sb[:, k, n * 512:(n + 1) * 512],
                            start=(k == 0), stop=False)
                    nc.tensor.matmul(
                        ps[:], ones_f32_row[:, 0:P],
                        bo_row[:, n * 512:(n + 1) * 512],
                        start=False, stop=True, skip_group_check=True)
                    nc.vector.tensor_add(
                        qrow[:, n * 512:(n + 1) * 512], ps[:],
                        x_own[:, rm, n * 512:(n + 1) * 512])
                for j in range(KC):
                    tp = s4pt.tile([P, P], BF16, tag="tp")
                    nc.tensor.transpose(tp[:], qrow[:, j * P:(j + 1) * P], ident[:])
                    nc.vector.tensor_copy(qT[:, j, rm * P:(rm + 1) * P], tp[:])

        att_cm.__exit__(None, None, None)
        x_cm.__exit__(None, None, None)

        # ================= Stage 5: cross K/V =============================
        ckv_cm = tc.tile_pool(name="ckv", bufs=1)
        ckv_pool = ckv_cm.__enter__()
        KTc = ckv_pool.tile([P, KC, S], BF16, tag="KTc")       # 2 MiB
        Vc = ckv_pool.tile([P, KC, D], BF16, tag="Vc")         # 2 MiB

        with tc.tile_pool(name="wkv", bufs=1) as wkv_pool, \
             tc.tile_pool(name="zT", bufs=1) as zT_pool, \
             tc.tile_pool(name="s5ps", bufs=4, space="PSUM") as s5ps:
            zT = zT_pool.tile([P, KC, S], BF16, tag="zT")      # 2 MiB
            for k in range(KC):
                nc.sync.dma_start_transpose(zT[:, k, :], z_d[:, k * P:(k + 1) * P])
            wkv_sb = wkv_pool.tile([P, KC, 2 * D], BF16, tag="wkv_sb")  # 4 MiB
            for k in range(KC):
                nc.sync.dma_start(wkv_sb[:, k, :], wkv_d[k * P:(k + 1) * P, :])
            # --- K_c^T ---
            for m in range(KC):
                for n in range(2):
                    ps = s5ps.tile([P, 512], FP32, tag="mm")
                    for k in range(KC):
                        nc.tensor.matmul(
                            ps[:], wkv_sb[:, k, m * P:(m + 1) * P],
                            zT[:, k, n * 512:(n + 1) * 512],
                            start=(k == 0), stop=(k == KC - 1))
                    nc.scalar.activation(KTc[:, m, n * 512:(n + 1) * 512], ps[:],
                                         AF.Identity, bias=bkc_pm[:, m:m + 1])
            # --- V_c (row-major) ---
            for si in range(KC):
                for n in range(2):
                    ps = s5ps.tile([P, 512], FP32, tag="mm")
                    for k in range(KC):
                        nc.tensor.matmul(
                            ps[:], zT[:, k, si * P:(si + 1) * P],
                            wkv_sb[:, k, D + n * 512:D + (n + 1) * 512],
                            start=(k == 0), stop=False)
                    nc.tensor.matmul(
                        ps[:], ones_f32_row[:, 0:P],
                        bvc_row[:, n * 512:(n + 1) * 512],
                        start=False, stop=True, skip_group_check=True)
                    nc.vector.tensor_copy(Vc[:, si, n * 512:(n + 1) * 512], ps[:])

        # ================= Stage 6: cross-attention =======================
        out_c = outc_pool.tile([P, RC, D], FP32, tag="out_c")  # 2 MiB

        with tc.tile_pool(name="s6", bufs=2) as s6, \
             tc.tile_pool(name="s6r", bufs=3) as s6r, \
             tc.tile_pool(name="s6ps", bufs=3, space="PSUM") as s6ps, \
             tc.tile_pool(name="s6pd", bufs=2, space="PSUM") as s6pd:
            Ec = s6.tile([P, KC, R], BF16, tag="Ec")           # 1 MiB
            for si in range(KC):
                ps = s6ps.tile([P, R], FP32, tag="sc")
                for k in range(KC):
                    nc.tensor.matmul(
                        ps[:], KTc[:, k, si * P:(si + 1) * P], qT[:, k, :],
                        start=(k == 0), stop=(k == KC - 1))
                nc.scalar.activation(Ec[:, si, :], ps[:], AF.Exp, scale=1.0 / 32.0)
            for rm in range(RC):
                pd = s6pd.tile([P, 1], FP32, tag="pd")
                for si in range(KC):
                    nc.tensor.matmul(
                        pd[:], Ec[:, si, rm * P:(rm + 1) * P], ones_bf_col[:],
                        start=(si == 0), stop=(si == KC - 1))
                recip = s6r.tile([P, 1], FP32, tag="recip")
                nc.vector.reciprocal(recip[:], pd[:])
                for n in range(2):
                    ps = s6ps.tile([P, R], FP32, tag="sc")
                    for si in range(KC):
                        nc.tensor.matmul(
                            ps[:], Ec[:, si, rm * P:(rm + 1) * P],
                            Vc[:, si, n * 512:(n + 1) * 512],
                            start=(si == 0), stop=(si == KC - 1))
                    nc.vector.tensor_scalar_mul(
                        out_c[:, rm, n * 512:(n + 1) * 512], ps[:], recip[:])

        ckv_cm.__exit__(None, None, None)
        qT_cm.__exit__(None, None, None)

        # ================= Stage 7: LN2 + transpose =======================
        h1_cm = tc.tile_pool(name="h1T", bufs=1)
        h1_pool = h1_cm.__enter__()
        ln2T_cm = tc.tile_pool(name="ln2T", bufs=1)
        ln2T_pool = ln2T_cm.__enter__()
        ln2T = ln2T_pool.tile([P, KC, R], BF16, tag="ln2T")    # 1 MiB

        with tc.tile_pool(name="s7", bufs=3) as s7, \
             tc.tile_pool(name="s7ps", bufs=4, space="PSUM") as s7ps:
            for rm in range(RC):
                stats = s7.tile([P, 2, 6], FP32, tag="stats")
                nc.vector.bn_stats(stats[:, 0, :], out_c[:, rm, 0:512])
                nc.vector.bn_stats(stats[:, 1, :], out_c[:, rm, 512:1024])
                mv = s7.tile([P, 2], FP32, tag="mv")
                nc.vector.bn_aggr(mv[:], stats[:])
                rstd = s7.tile([P, 1], FP32, tag="rstd")
                nc.scalar.activation(rstd[:], mv[:, 1:2], AF.Sqrt, bias=eps_t[:])
                nc.vector.reciprocal(rstd[:], rstd[:])
                ln_row = s7.tile([P, D], BF16, tag="ln_row")
                nc.vector.tensor_scalar(
                    out=ln_row[:], in0=out_c[:, rm, :],
                    scalar1=mv[:, 0:1], scalar2=rstd[:],
                    op0=mybir.AluOpType.subtract, op1=mybir.AluOpType.mult,
                )
                for j in range(KC):
                    tp = s7ps.tile([P, P], BF16, tag="tp")
                    nc.tensor.transpose(tp[:], ln_row[:, j * P:(j + 1) * P], ident[:])
                    nc.vector.tensor_copy(ln2T[:, j, rm * P:(rm + 1) * P], tp[:])

        # ================= Stage 8: MLP up + gelu =========================
        h1T = h1_pool.tile([P, HC, R], BF16, tag="h1T")        # 4 MiB

        with tc.tile_pool(name="wm1", bufs=1) as wm1_pool, \
             tc.tile_pool(name="s8ps", bufs=4, space="PSUM") as s8ps:
            wm1_sb = wm1_pool.tile([P, KC, H], BF16, tag="wm1_sb")  # 8 MiB
            for k in range(KC):
                nc.sync.dma_start(wm1_sb[:, k, :], wm1_d[k * P:(k + 1) * P, :])
            for hm in range(HC):
                ps = s8ps.tile([P, R], FP32, tag="mm")
                for k in range(KC):
                    nc.tensor.matmul(
                        ps[:], wm1_sb[:, k, hm * P:(hm + 1) * P], ln2T[:, k, :],
                        start=(k == 0), stop=(k == KC - 1))
                nc.scalar.activation(h1T[:, hm, :], ps[:], AF.Gelu_apprx_tanh,
                                     bias=bm1_pm[:, hm:hm + 1])

        ln2T_cm.__exit__(None, None, None)

        # ================= Stage 9: MLP down + final ======================
        with tc.tile_pool(name="wm2", bufs=1) as wm2_pool, \
             tc.tile_pool(name="s9", bufs=3) as s9, \
             tc.tile_pool(name="s9ps", bufs=3, space="PSUM") as s9ps:
            wm2_sb = wm2_pool.tile([P, HC, D], BF16, tag="wm2_sb")  # 8 MiB
            for k in range(HC):
                nc.sync.dma_start(wm2_sb[:, k, :], wm2_d[k * P:(k + 1) * P, :])
            for rm in range(RC):
                for n in range(2):
                    ps = s9ps.tile([P, 512], FP32, tag="mm")
                    for k in range(HC):
                        nc.tensor.matmul(
                            ps[:], h1T[:, k, rm * P:(rm + 1) * P],
                            wm2_sb[:, k, n * 512:(n + 1) * 512],
                            start=(k == 0), stop=False)
                    nc.tensor.matmul(
                        ps[:], ones_f32_row[:, 0:P],
                        bm2_row[:, n * 512:(n + 1) * 512],
                        start=False, stop=True, skip_group_check=True)
                    fin = s9.tile([P, 512], FP32, tag="fin")
                    nc.vector.tensor_add(fin[:], ps[:],
                                         out_c[:, rm, n * 512:(n + 1) * 512])
                    nc.sync.dma_start(
                        out_d[rm * P:(rm + 1) * P, n * 512:(n + 1) * 512], fin[:])

        h1_cm.__exit__(None, None, None)
        outc_cm.__exit__(None, None, None)

    _split_multi_waits(nc)
    return nc


_NC = None


def _get_nc():
    global _NC
    if _NC is None:
        _NC = build()
    return _NC


def kernel(x, z, g1, b1, w_qkv, b_qkv, w_o, b_o, w_kv, b_kv, g2, b2,
           w_m1, b_m1, w_m2, b_m2):
    x = np.asarray(x, np.float32)
    z = np.asarray(z, np.float32)
    bf = ml_dtypes.bfloat16

    # fold LN gains into the following weights (exact)
    wqkv_f = (np.asarray(g1, np.float32)[:, None] * np.asarray(w_qkv, np.float32))
    bqkv_f = (np.asarray(b1, np.float32) @ np.asarray(w_qkv, np.float32)
              + np.asarray(b_qkv, np.float32))
    wm1_f = (np.asarray(g2, np.float32)[:, None] * np.asarray(w_m1, np.float32))
    bm1_f = (np.asarray(b2, np.float32) @ np.asarray(w_m1, np.float32)
             + np.asarray(b_m1, np.float32))

    shared = {
        "wqkv": wqkv_f.astype(bf),
        "wo": np.asarray(w_o, np.float32).astype(bf),
        "wkv": np.asarray(w_kv, np.float32).astype(bf),
        "wm1": wm1_f.astype(bf),
        "wm2": np.asarray(w_m2, np.float32).astype(bf),
        "bqkv": np.ascontiguousarray(bqkv_f, dtype=np.float32),
        "bkv": np.asarray(b_kv, np.float32),
        "bo": np.asarray(b_o, np.float32),
        "bm1": np.ascontiguousarray(bm1_f, dtype=np.float32),
        "bm2": np.asarray(b_m2, np.float32),
    }

    B = x.shape[0]
    in_maps = []
    for c in range(2 * B):
        b, half = c // 2, c % 2
        xb = x[b]
        if half == 1:  # rotate so own query rows are rows 0:R
            xb = np.concatenate([xb[R:], xb[:R]], axis=0)
        in_maps.append({
            "x": np.ascontiguousarray(xb, dtype=np.float32),
            "z": z[b].astype(bf),
            **shared,
        })

    res = run_bass_kernel_spmd(_get_nc(), in_maps, core_ids=list(range(2 * B)))

    out = np.empty((B, S, D), np.float32)
    for c in range(2 * B):
        b, half = c // 2, c % 2
        out[b, half * R:(half + 1) * R, :] = res.results[c]["out"]
    return out
